# revision 1
# baseline (speedup 1.0000x reference)
"""Causal self-attention on 8 Trainium2 NeuronCores (Bass/Tile).

Problem: B=2, S=2048, D=1024, H=16 heads (hd=64), fp32 in/out.

Sharding (SPMD, same NEFF on 8 cores, different data):
  core c -> batch b = c//4, head-group g = c%4 (4 heads = 256 cols of wq/wk/wv,
  256 rows of wo). Each core computes its 4 heads' attention plus the partial
  output projection. Host sums the 4 partials per batch and adds bo.

Per-core dataflow (everything stays transposed so no on-device transposes):
  qT/kT = matmul(lhsT=w[d,j], rhs=xT[d,s]) -> [j, s]
  v     = matmul(lhsT=xT[d,s], rhs=wv[d,j]) -> [s, j]
  scoresT[s_k, s_q] = matmul(lhsT=kT[j, s_k], rhs=qT[j, s_q])  (K=64, two heads
      row-packed into the 128-row PE array via tile_position)
  causal mask: an extra accumulation matmul (lhsT=identity, rhs=mask_d const)
      adds -1e5 to the invalid triangle of diagonal blocks, in PSUM.
  P = exp(scores/8)  (ScalarE, PSUM->SBUF, float32r out)
  yT[j, s_q] (+ row of column sums via a ones column in the stationary)
      = matmul(lhsT=[v|ones], rhs=P)  (K=128)
  normalize by sums: DVE reciprocal of the sums row, broadcast across
      partitions with a PE matmul (lhsT = row-64-ones constant), DVE mul.
      The odd head's rows hop to partitions 64:128 via a SBUF-SBUF DMA.
  outT[e, s] = matmul(lhsT=wo[j, e], rhs=yT[j, s])  (accumulate over j)

Matmuls run in float32r (full-rate PE, ~1.6e-4 rel err vs 4x-slower fp32).
Diagonal blocks are column-narrowed: for key-block offset d (0..3) only query
columns >= 128d can be unmasked, so scores/mask/exp/AV all operate on the
trailing 512-128d columns.
"""

import numpy as np

import concourse.bass as bass
import concourse.tile as tile
from concourse import bacc, mybir
from concourse.bass_utils import run_bass_kernel_spmd

P = 128
B, S, D, H, HD = 2, 2048, 1024, 16, 64
JH = 256          # head-dim columns per core (4 heads x 64)
KT = D // P       # 8 contraction tiles for the projections
QC = 512          # query-chunk (matmul moving free dim)
NQC = S // QC     # 4
NKB = S // P      # 16 key blocks
MASKVAL = -1.0e5
F32 = mybir.dt.float32
F32R = mybir.dt.float32r
AF = mybir.ActivationFunctionType

_NC = None


def build(repeats: int = 1, num_devices: int = 8):
    nc = bacc.Bacc(
        "TRN2", target_bir_lowering=False, debug=False, num_devices=num_devices
    )

    xT_d = nc.dram_tensor("xT", [D, S], F32, kind="ExternalInput").ap()
    wq_d = nc.dram_tensor("wq", [D, JH], F32, kind="ExternalInput").ap()
    wk_d = nc.dram_tensor("wk", [D, JH], F32, kind="ExternalInput").ap()
    wv_d = nc.dram_tensor("wv", [D, JH], F32, kind="ExternalInput").ap()
    wo_d = nc.dram_tensor("wo", [JH, D], F32, kind="ExternalInput").ap()
    bq_d = nc.dram_tensor("bq", [JH], F32, kind="ExternalInput").ap()
    bk_d = nc.dram_tensor("bk", [JH], F32, kind="ExternalInput").ap()
    bv_d = nc.dram_tensor("bv", [JH], F32, kind="ExternalInput").ap()
    outT_d = nc.dram_tensor("outT", [D, S], F32, kind="ExternalOutput").ap()

    xT_re = xT_d.rearrange("(o p) s -> p o s", p=P)      # [128, 8, 2048]
    wq_re = wq_d.rearrange("(o p) j -> p o j", p=P)      # [128, 8, 256]
    wk_re = wk_d.rearrange("(o p) j -> p o j", p=P)
    wv_re = wv_d.rearrange("(o p) j -> p o j", p=P)
    wo_re = wo_d.rearrange("(o p) e -> p o e", p=P)      # [128, 2, 1024]
    bq_re = bq_d.rearrange("(t p) -> p t", p=P)          # [128, 2]
    bk_re = bk_d.rearrange("(t p) -> p t", p=P)
    outT_re = outT_d.rearrange("(o p) s -> p o s", p=P)  # [128, 8, 2048]

    with tile.TileContext(nc) as tc:
        with (
            tc.tile_pool(name="persist", bufs=1) as persist,
            # shared big PSUM pool: [128, 2, 512] = 2 banks, x3 bufs = 6 banks.
            # Used for QKV accumulators, score pairs, and out-proj accumulators.
            tc.tile_pool(name="ps_a", bufs=2, space="PSUM") as ps_a,
            tc.tile_pool(name="ps_o", bufs=2, space="PSUM") as ps_o,
            tc.tile_pool(name="ps_yt", bufs=2, space="PSUM") as ps_yt,
        ):
            # ---------------- persistent SBUF ----------------
            qT = persist.tile([P, 2, S], F32R, tag="qT")
            kT = persist.tile([P, 2, S], F32R, tag="kT")
            # per (k-block, head): [v(0:64) | ones(64)]
            vsb = persist.tile([P, NKB, 4, 65], F32R, tag="vsb")
            yT = persist.tile([P, 2, S], F32R, tag="yT")
            wo_r = persist.tile([P, 2, D], F32R, tag="wo_r")
            bq_sb = persist.tile([P, 2], F32, tag="bq")
            bk_sb = persist.tile([P, 2], F32, tag="bk")
            e0_r = persist.tile([P, P], F32R, tag="e0r")
            bvpad_r = persist.tile([P, JH], F32R, tag="bvpadr")
            ident_r = persist.tile([P, P], F32R, tag="identr")
            mask_r = persist.tile([P, 4, QC], F32R, tag="maskr")
            # row-64-ones stationary + dedicated reciprocal-row tiles for the
            # PE-based sums broadcast (rows other than 64 stay zero forever)
            e64_r = persist.tile([P, P], F32R, tag="e64r")
            rtr0 = persist.tile([P, QC], F32R, tag="rtr0")
            rtr1 = persist.tile([P, QC], F32R, tag="rtr1")

            nc.sync.dma_start(bq_sb[:], bq_re)
            nc.sync.dma_start(bk_sb[:], bk_re)

            # fp32 protos in a transient pool (memset/affine ok for fp32; the
            # f32r copies of them must come from DVE/ACT producers)
            with tc.tile_pool(name="initp", bufs=1) as initp:
                vproto = initp.tile([P, 2, 65], F32, tag="vproto")
                e0_f = initp.tile([P, P], F32, tag="e0f")
                bvpad_f = initp.tile([P, JH], F32, tag="bvpadf")
                ident_f = initp.tile([P, P], F32, tag="identf")
                mask_f = initp.tile([P, 4, QC], F32, tag="maskf")

                # e0: row 0 = ones, rest 0  (for the +bv accumulation matmul)
                nc.gpsimd.memset(e0_f[:], 0.0)
                nc.gpsimd.memset(e0_f[0:1, :], 1.0)
                nc.vector.tensor_copy(e0_r[:], e0_f[:])
                # e64: row 64 = ones (sums-broadcast matmul); zero recip tiles
                nc.gpsimd.memset(e0_f[:], 0.0)
                nc.gpsimd.memset(e0_f[64:65, :], 1.0)
                nc.vector.tensor_copy(e64_r[:], e0_f[:])
                nc.gpsimd.memset(e0_f[:], 0.0)
                nc.vector.tensor_copy(rtr0[:], e0_f[:, 0:1].to_broadcast((P, QC)))
                nc.vector.tensor_copy(rtr1[:], e0_f[:, 0:1].to_broadcast((P, QC)))
                # re-make e0 row 0 ones (it was reused as scratch above)
                nc.gpsimd.memset(e0_f[0:1, :], 1.0)
                nc.gpsimd.memset(bvpad_f[:], 0.0)
                nc.sync.dma_start(bvpad_f[0:1, :], bv_d[None, :])
                nc.vector.tensor_copy(bvpad_r[:], bvpad_f[:])

                # identity (for mask-add matmuls): 1.0 only on the diagonal
                nc.gpsimd.memset(ident_f[:], 1.0)
                nc.gpsimd.affine_select(
                    out=ident_f[:], in_=ident_f[:],
                    compare_op=mybir.AluOpType.is_equal,
                    fill=0.0, base=0,
                    pattern=[[-1, P]], channel_multiplier=1,
                )
                nc.vector.tensor_copy(ident_r[:], ident_f[:])

                # mask_d[kk, qq] = 0 where qq - kk - 128d >= 0 else MASKVAL
                nc.gpsimd.memset(mask_f[:], 0.0)
                for d in range(4):
                    nc.gpsimd.affine_select(
                        out=mask_f[:, d, :], in_=mask_f[:, d, :],
                        compare_op=mybir.AluOpType.is_ge,
                        fill=MASKVAL, base=-128 * d,
                        pattern=[[1, QC]], channel_multiplier=-1,
                    )
                nc.vector.tensor_copy(mask_r[:], mask_f[:])

                # v prototype row: ones at col 64 ([v|ones] for every head)
                nc.gpsimd.memset(vproto[:], 0.0)
                nc.gpsimd.memset(vproto[:, 0, 64:65], 1.0)
                nc.vector.tensor_copy(
                    vsb[:],
                    vproto[:, None, 0:1, :].to_broadcast((P, NKB, 4, 65)),
                )

            for _rep in range(repeats):
                # ---------------- phase 1: load, round, project ----------------
                with (
                    tc.tile_pool(name="ph1", bufs=1) as ph1,
                    tc.tile_pool(name="xstage", bufs=2) as xstage,
                    tc.tile_pool(name="wstage", bufs=2) as wstage_pool,
                ):
                    wq_r = ph1.tile([P, KT, JH], F32R, tag="wq_r")
                    wk_r = ph1.tile([P, KT, JH], F32R, tag="wk_r")
                    wv_r = ph1.tile([P, KT, JH], F32R, tag="wv_r")
                    xT_r = ph1.tile([P, KT, S], F32R, tag="xT_r")

    # x k-tiles 0..3 first (sweep A's critical input), then weights,
                    # then the rest of x.
                    xs_tiles = []
                    for kt in range(4):
                        xs = xstage.tile([P, S], F32, tag="xs")
                        nc.sync.dma_start(xs[:], xT_re[:, kt, :])
                        xs_tiles.append(xs)
                    for w_re, w_r in ((wk_re, wk_r), (wq_re, wq_r), (wv_re, wv_r)):
                        st = wstage_pool.tile([P, KT, JH], F32, tag="wst")
                        nc.sync.dma_start(st[:], w_re)
                        nc.vector.tensor_copy(w_r[:], st[:])
                    wost = wstage_pool.tile([P, KT, JH], F32, tag="wst")
                    nc.sync.dma_start(
                        wost[:].rearrange("p a b -> p (a b)").rearrange(
                            "p (a b) -> p a b", a=2
                        ),
                        wo_re,
                    )
                    nc.vector.tensor_copy(
                        wo_r[:],
                        wost[:].rearrange("p a b -> p (a b)").rearrange(
                            "p (a b) -> p a b", a=2
                        ),
                    )

                    for kt in range(KT):
                        if kt < 4:
                            xs = xs_tiles[kt]
                        else:
                            xs = xstage.tile([P, S], F32, tag="xs")
                            nc.sync.dma_start(xs[:], xT_re[:, kt, :])
                        nc.vector.tensor_copy(xT_r[:, kt, :], xs[:])

                    # Projections with split contraction, two sweeps:
                    #  sweep A: x k-tiles 0..3 for every group, evacuated by
                    #           ScalarE straight into the f32r destination
                    #           (with bias for K/Q) while x 4..7 still streams;
                    #  sweep B: x k-tiles 4..7, combined into the destination
                    #           with an in-place DVE add.
                    HKT = KT // 2

                    def kq_groups():
                        for jt in range(2):
                            for w_r, bias_sb, dst in (
                                (wk_r, bk_sb, kT),
                                (wq_r, bq_sb, qT),
                            ):
                                for sc in range(NQC):
                                    yield jt, w_r, bias_sb, dst, sc

                    for jt, w_r, bias_sb, dst, sc in kq_groups():
                        acc = ps_o.tile([P, QC], F32, tag="o")
                        for kt in range(HKT):
                            nc.tensor.matmul(
                                acc,
                                w_r[:, kt, jt * P : (jt + 1) * P],
                                xT_r[:, kt, sc * QC : (sc + 1) * QC],
                                start=(kt == 0),
                                stop=(kt == HKT - 1),
                            )
                        nc.scalar.activation(
                            dst[:, jt, sc * QC : (sc + 1) * QC],
                            acc,
                            AF.Identity,
                            bias=bias_sb[:, jt : jt + 1],
                        )
                    for st_i in range(NKB):
                        acc = ps_o.tile([P, QC], F32, tag="o")
                        va = acc[:, 0:JH]
                        for kt in range(HKT):
                            nc.tensor.matmul(
                                va,
                                xT_r[:, kt, st_i * P : (st_i + 1) * P],
                                wv_r[:, kt, :],
                                start=(kt == 0),
                                stop=(kt == HKT - 1),
                            )
                        nc.scalar.copy(
                            vsb[:, st_i, :, 0:64],
                            va.rearrange("p (h j) -> p h j", h=4),
                        )

                    for jt, w_r, bias_sb, dst, sc in kq_groups():
                        acc = ps_o.tile([P, QC], F32, tag="o")
                        for kt in range(HKT, KT):
                            nc.tensor.matmul(
                                acc,
                                w_r[:, kt, jt * P : (jt + 1) * P],
                                xT_r[:, kt, sc * QC : (sc + 1) * QC],
                                start=(kt == HKT),
                                stop=(kt == KT - 1),
                            )
                        dsl = dst[:, jt, sc * QC : (sc + 1) * QC]
                        nc.vector.tensor_add(dsl, acc, dsl)
                    for st_i in range(NKB):
                        acc = ps_o.tile([P, QC], F32, tag="o")
                        va = acc[:, 0:JH]
                        for kt in range(HKT, KT):
                            nc.tensor.matmul(
                                va,
                                xT_r[:, kt, st_i * P : (st_i + 1) * P],
                                wv_r[:, kt, :],
                                start=(kt == HKT),
                                stop=False,
                            )
                        nc.tensor.matmul(
                            va, e0_r[:], bvpad_r[:], start=False, stop=True
                        )
                        vsl = vsb[:, st_i, :, 0:64]
                        nc.vector.tensor_add(
                            vsl, va.rearrange("p (h j) -> p h j", h=4), vsl
                        )

                # ------------- phase 2: attention + out-proj per q-chunk -------------
                with (
                    tc.tile_pool(name="pt_pool", bufs=4) as pt_pool,
                    tc.tile_pool(name="recip", bufs=2) as recip_pool,
                    tc.tile_pool(name="ostage", bufs=2) as ostage,
                ):
                    for qc in range(NQC):
                        for pair in range(2):
                            nkb = 4 * (qc + 1)
                            y0 = ps_yt.tile([P, QC], F32, tag="yt")
                            y1 = ps_yt.tile([P, QC], F32, tag="yt")
                            for kb in range(nkb):
                                d = kb - 4 * qc  # >= 0 on diagonal blocks
                                n_d = QC - 128 * d if d > 0 else QC
                                q_off = qc * QC + (QC - n_d)
                                sc_ps = ps_a.tile([P, 2, QC], F32, tag="a")
                                for he in range(2):
                                    nc.tensor.matmul(
                                        sc_ps[:, he, 0:n_d],
                                        kT[64 * he : 64 * he + 64, pair,
                                           kb * P : (kb + 1) * P],
                                        qT[64 * he : 64 * he + 64, pair,
                                           q_off : q_off + n_d],
                                        start=True,
                                        stop=(d < 0),
                                        tile_position=(64 * he, 0),
                                    )
                                if d >= 0:
                                    # add causal mask in PSUM via identity matmul
                                    for he in range(2):
                                        nc.tensor.matmul(
                                            sc_ps[:, he, 0:n_d],
                                            ident_r[:],
                                            mask_r[:, d, QC - n_d :],
                                            start=False,
                                            stop=True,
                                        )
                                pt = pt_pool.tile([P, 2, QC], F32R, tag="pt")
                                nc.scalar.activation(
                                    pt[:, :, 0:n_d],
                                    sc_ps[:, :, 0:n_d],
                                    AF.Exp,
                                    scale=0.125,
                                )
                                h0 = 2 * pair
                                for he, yps in ((0, y0), (1, y1)):
                                    nc.tensor.matmul(
                                        yps[0:65, QC - n_d :],
                                        vsb[:, kb, h0 + he, :],
                                        pt[:, he, 0:n_d],
                                        start=(kb == 0),
                                        stop=(kb == nkb - 1),
                                    )
                            # evacuate the AV accumulators to SBUF right away so
                            # the PSUM slots free for the next pair's AV chain;
                            # normalize from the SBUF copies off the hot path.
                            ysg0 = recip_pool.tile([P, QC], F32, tag="ysg")
                            ysg1 = recip_pool.tile([P, QC], F32, tag="ysg")
                            nc.vector.tensor_copy(ysg0[0:65, :], y0[0:65, :])
                            nc.vector.tensor_copy(ysg1[0:65, :], y1[0:65, :])
                            # normalize: y /= column-sums (sums at row 64).
                            # Broadcast the reciprocal row across partitions
                            # with a PE matmul (lhsT = row-64-ones) instead of
                            # DMA hop + gpsimd: much shorter critical path.
                            with nc.allow_low_precision(
                                reason="f32r reciprocal row for sums broadcast"
                            ):
                                nc.vector.reciprocal(
                                    rtr0[64:65, :], ysg0[64:65, :]
                                )
                                nc.vector.reciprocal(
                                    rtr1[64:65, :], ysg1[64:65, :]
                                )
                            rbp0 = ps_o.tile([P, QC], F32, tag="o")
                            nc.tensor.matmul(
                                rbp0[:], e64_r[:], rtr0[:], start=True, stop=True
                            )
                            nc.vector.tensor_mul(
                                yT[0:64, pair, qc * QC : (qc + 1) * QC],
                                ysg0[0:64, :],
                                rbp0[0:64, :],
                            )
                            # odd head: DMA-hop the unnormalized y to partitions
                            # 64:128 in parallel with the reciprocal chain, then
                            # multiply directly into yT (DVE mul is the f32r
                            # rounding producer).
                            ymid = recip_pool.tile([P, QC], F32, tag="ymid")
                            nc.sync.dma_start(ymid[64:128, :], ysg1[0:64, :])
                            rbp1 = ps_o.tile([P, QC], F32, tag="o")
                            nc.tensor.matmul(
                                rbp1[:], e64_r[:], rtr1[:], start=True, stop=True
                            )
                            nc.vector.tensor_mul(
                                yT[64:128, pair, qc * QC : (qc + 1) * QC],
                                ymid[64:128, :],
                                rbp1[64:128, :],
                            )

                        # out-projection for this q-chunk
                        ot = ostage.tile([P, KT, QC], F32, tag="ot")
                        for et in range(KT):
                            acc = ps_o.tile([P, QC], F32, tag="o")
                            for pair in range(2):
                                nc.tensor.matmul(
                                    acc,
                                    wo_r[:, pair, et * P : (et + 1) * P],
                                    yT[:, pair, qc * QC : (qc + 1) * QC],
                                    start=(pair == 0),
                                    stop=(pair == 1),
                                )
                            nc.vector.tensor_copy(ot[:, et, :], acc)
                        nc.sync.dma_start(
                            outT_re[:, :, qc * QC : (qc + 1) * QC], ot[:]
                        )

    nc.compile()
    return nc


def _get_nc():
    global _NC
    if _NC is None:
        _NC = build()
    return _NC


def kernel(x, wq, bq, wk, bk, wv, bv, wo, bo, **run_kwargs):
    x = np.asarray(x, dtype=np.float32)
    wq = np.asarray(wq, dtype=np.float32)
    bq = np.asarray(bq, dtype=np.float32)
    wk = np.asarray(wk, dtype=np.float32)
    bk = np.asarray(bk, dtype=np.float32)
    wv = np.asarray(wv, dtype=np.float32)
    bv = np.asarray(bv, dtype=np.float32)
    wo = np.asarray(wo, dtype=np.float32)
    bo = np.asarray(bo, dtype=np.float32)

    nc = _get_nc()
    in_maps = []
    for c in range(8):
        b, g = divmod(c, 4)
        jsl = slice(JH * g, JH * (g + 1))
        in_maps.append(
            {
                "xT": np.ascontiguousarray(x[b].T),
                "wq": np.ascontiguousarray(wq[:, jsl]),
                "wk": np.ascontiguousarray(wk[:, jsl]),
                "wv": np.ascontiguousarray(wv[:, jsl]),
                "wo": np.ascontiguousarray(wo[jsl, :]),
                "bq": np.ascontiguousarray(bq[jsl]),
                "bk": np.ascontiguousarray(bk[jsl]),
                "bv": np.ascontiguousarray(bv[jsl]),
            }
        )
    res = run_bass_kernel_spmd(nc, in_maps, core_ids=list(range(8)), **run_kwargs)
    outs = [r["outT"] for r in res.results]
    y = np.empty((B, S, D), dtype=np.float32)
    for b in range(B):
        acc = outs[4 * b] + outs[4 * b + 1] + outs[4 * b + 2] + outs[4 * b + 3]
        y[b] = acc.T + bo[None, :]
    if run_kwargs:
        kernel.last_result = res
    return y



# revision 2
# speedup vs baseline: 1.2752x; 1.2752x over previous
"""Causal self-attention on 8 Trainium2 NeuronCores (Bass/Tile) — v2.

Problem: B=2, S=2048, D=1024, H=16 heads (hd=64), fp32 in/out.

Sharding (SPMD, same NEFF on 8 cores, different data):
  core c -> batch b = c//4, head-group g = c%4 (4 heads = 256 cols of wq/wk/wv,
  256 rows of wo). Each core computes its 4 heads' attention plus the partial
  output projection. Host sums the 4 partials per batch and adds bo.

v2 changes vs v1 (196975ns sim):
  * x / wq / wk / wv / wo are converted to bf16 on the host: input DMA halves
    and the DMA'd tiles feed the PE directly (bf16 moving operand is full-rate
    at any output width, so no on-chip fp32->f32r conversion copies at all).
  * qT/kT/vsb/P/yT/output all bf16 (validated 4.0e-3 rel err vs the 2e-2 gate).
  * causal-mask add narrowed to the single 128-col triangle tile per diagonal
    key block (the rest of the block needs no mask), in bf16.
  * bv folded into the vsb init (broadcast once via a row-select matmul)
    instead of a per-key-block accumulation matmul.
  * software pipelining: K/Q sweep A runs kt-major across 8 PSUM accumulators
    to track the x DMA stream; sweep B + the V projection are interleaved with
    the attention q-chunks; the softmax normalize of chunk qc is deferred past
    the projection work of chunk qc+1 so the PE never waits on the DVE
    reciprocal chain; output DMA is streamed in 2-row-block pieces.

Per-core dataflow (everything stays transposed so no on-device transposes):
  kT/qT = matmul(lhsT=w[d,j], rhs=xT[d,s]) -> [j, s]   (bias via ACT evac)
  v     = matmul(lhsT=xT[d,s], rhs=wv[d,j]) -> [s, j]  (+bv pre-filled in vsb)
  scoresT[s_k, s_q] = matmul(lhsT=kT[j, s_k], rhs=qT[j, s_q])  (K=64, two heads
      row-packed into the 128-row PE array via tile_position)
  causal mask: one extra accumulation matmul (lhsT=identity, rhs=tri const)
      adds -1e5 to the 128-wide triangle tile of diagonal blocks, in PSUM.
  P = exp(scores/8)  (ScalarE, PSUM->SBUF, bf16 out)
  yT[j, s_q] (+ row of column sums via a ones column in the stationary)
      = matmul(lhsT=[v|ones], rhs=P)  (K=128)
  normalize by sums: DVE reciprocal of the sums row, broadcast across
      partitions with a PE matmul (lhsT = row-64-ones constant), DVE mul.
      The odd head's rows hop to partitions 64:128 via a SBUF-SBUF DMA.
  outT[e, s] = matmul(lhsT=wo[j, e], rhs=yT[j, s])  (accumulate over j)
"""

import numpy as np
import ml_dtypes

import concourse.bass as bass
import concourse.tile as tile
from concourse import bacc, mybir
from concourse.bass_utils import run_bass_kernel_spmd

P = 128
B, S, D, H, HD = 2, 2048, 1024, 16, 64
JH = 256          # head-dim columns per core (4 heads x 64)
KT = D // P       # 8 contraction tiles for the projections
QC = 512          # query-chunk (matmul moving free dim)
NQC = S // QC     # 4
NKB = S // P      # 16 key blocks
MASKVAL = -1.0e5
F32 = mybir.dt.float32
F32R = mybir.dt.float32r
BF = mybir.dt.bfloat16
AF = mybir.ActivationFunctionType
BF16NP = ml_dtypes.bfloat16

_NC = None


def build(repeats: int = 1, num_devices: int = 8):
    nc = bacc.Bacc(
        "TRN2", target_bir_lowering=False, debug=False, num_devices=num_devices
    )

    xT_d = nc.dram_tensor("xT", [D, S], BF, kind="ExternalInput").ap()
    wq_d = nc.dram_tensor("wq", [D, JH], BF, kind="ExternalInput").ap()
    wk_d = nc.dram_tensor("wk", [D, JH], BF, kind="ExternalInput").ap()
    wv_d = nc.dram_tensor("wv", [D, JH], BF, kind="ExternalInput").ap()
    wo_d = nc.dram_tensor("wo", [JH, D], BF, kind="ExternalInput").ap()
    bq_d = nc.dram_tensor("bq", [JH], F32, kind="ExternalInput").ap()
    bk_d = nc.dram_tensor("bk", [JH], F32, kind="ExternalInput").ap()
    bv_d = nc.dram_tensor("bv", [JH], F32, kind="ExternalInput").ap()
    outT_d = nc.dram_tensor("outT", [D, S], BF, kind="ExternalOutput").ap()

    xT_re = xT_d.rearrange("(o p) s -> p o s", p=P)      # [128, 8, 2048]
    wq_re = wq_d.rearrange("(o p) j -> p o j", p=P)      # [128, 8, 256]
    wk_re = wk_d.rearrange("(o p) j -> p o j", p=P)
    wv_re = wv_d.rearrange("(o p) j -> p o j", p=P)
    wo_re = wo_d.rearrange("(o p) e -> p o e", p=P)      # [128, 2, 1024]
    bq_re = bq_d.rearrange("(t p) -> p t", p=P)          # [128, 2]
    bk_re = bk_d.rearrange("(t p) -> p t", p=P)
    outT_re = outT_d.rearrange("(o p) s -> p o s", p=P)  # [128, 8, 2048]

    with tile.TileContext(nc) as tc:
        with (
            tc.tile_pool(name="persist", bufs=1) as persist,
            tc.tile_pool(name="ps_a", bufs=2, space="PSUM") as ps_a,     # 4 banks
            tc.tile_pool(name="ps_o", bufs=2, space="PSUM") as ps_o,     # 2 banks
            tc.tile_pool(name="ps_yt", bufs=2, space="PSUM") as ps_yt,   # 2 banks
            tc.tile_pool(name="pt_pool", bufs=4) as pt_pool,
            tc.tile_pool(name="recip", bufs=6) as recip_pool,
            tc.tile_pool(name="ostage", bufs=2) as ostage,
        ):
            # ---------------- persistent SBUF ----------------
            xsb = persist.tile([P, KT, S], BF, tag="xsb")
            wk_sb = persist.tile([P, KT, JH], BF, tag="wk")
            wq_sb = persist.tile([P, KT, JH], BF, tag="wq")
            wv_sb = persist.tile([P, KT, JH], BF, tag="wv")
            wo_sb = persist.tile([P, 2, D], BF, tag="wo")
            qT = persist.tile([P, 2, S], BF, tag="qT")
            kT = persist.tile([P, 2, S], BF, tag="kT")
            vsb = persist.tile([P, NKB, 4, 65], BF, tag="vsb")
            yT = persist.tile([P, 2, S], BF, tag="yT")
            bq_sb = persist.tile([P, 2], F32, tag="bq")
            bk_sb = persist.tile([P, 2], F32, tag="bk")
            ident_b = persist.tile([P, P], BF, tag="identb")
            mask_b = persist.tile([P, P], BF, tag="maskb")
            e0_b = persist.tile([P, P], BF, tag="e0b")
            bvpad_b = persist.tile([P, JH], BF, tag="bvpadb")
            e64_r = persist.tile([P, P], F32R, tag="e64r")
            # reciprocal rows for the PE sums-broadcast; rows != 64 must be 0
            rtr = persist.tile([P, 2, 2, QC], F32R, tag="rtr")

            # ------- DMA issue order: first-compute-first -------
            nc.sync.dma_start(wk_sb[:], wk_re)
            nc.sync.dma_start(xsb[:, 0, :], xT_re[:, 0, :])
            nc.sync.dma_start(bq_sb[:], bq_re)
            nc.sync.dma_start(bk_sb[:], bk_re)
            nc.sync.dma_start(wq_sb[:], wq_re)
            for kt in range(1, 4):
                nc.sync.dma_start(xsb[:, kt, :], xT_re[:, kt, :])
            nc.sync.dma_start(wv_sb[:], wv_re)
            for kt in range(4, KT):
                nc.sync.dma_start(xsb[:, kt, :], xT_re[:, kt, :])
            nc.sync.dma_start(wo_sb[:], wo_re)

            # ---------------- constants ----------------
            with tc.tile_pool(name="initp", bufs=1) as initp:
                pf = initp.tile([P, P], F32, tag="pf")
                vproto = initp.tile([P, 4, 65], F32, tag="vproto")
                bvpad_f = initp.tile([P, JH], F32, tag="bvpadf")

                # identity (for mask-add matmuls)
                nc.gpsimd.memset(pf[:], 1.0)
                nc.gpsimd.affine_select(
                    out=pf[:], in_=pf[:],
                    compare_op=mybir.AluOpType.is_equal,
                    fill=0.0, base=0,
                    pattern=[[-1, P]], channel_multiplier=1,
                )
                nc.vector.tensor_copy(ident_b[:], pf[:])

                # triangle mask tile: MASKVAL where qq < kk (within-block)
                nc.gpsimd.memset(pf[:], 0.0)
                nc.gpsimd.affine_select(
                    out=pf[:], in_=pf[:],
                    compare_op=mybir.AluOpType.is_ge,
                    fill=MASKVAL, base=0,
                    pattern=[[1, P]], channel_multiplier=-1,
                )
                nc.vector.tensor_copy(mask_b[:], pf[:])

                # e64 (row 64 ones) for the sums broadcast; zero rtr rows
                nc.gpsimd.memset(pf[:], 0.0)
                nc.vector.tensor_copy(
                    rtr[:], pf[:, 0:1].to_broadcast((P, 2, 2, QC))
                )
                nc.gpsimd.memset(pf[64:65, :], 1.0)
                nc.vector.tensor_copy(e64_r[:], pf[:])

                # e0 (row 0 ones) for the bv broadcast matmul
                nc.gpsimd.memset(pf[:], 0.0)
                nc.gpsimd.memset(pf[0:1, :], 1.0)
                nc.vector.tensor_copy(e0_b[:], pf[:])
                nc.gpsimd.memset(bvpad_f[:], 0.0)
                nc.sync.dma_start(bvpad_f[0:1, :], bv_d[None, :])
                nc.vector.tensor_copy(bvpad_b[:], bvpad_f[:])

                # vsb prototype: [bv-broadcast | ones] per head, then fan out
                nc.gpsimd.memset(vproto[:], 0.0)
                nc.gpsimd.memset(vproto[:, :, 64:65], 1.0)
                psbv = ps_o.tile([P, JH], F32, tag="o")
                nc.tensor.matmul(psbv, e0_b[:], bvpad_b[:], start=True, stop=True)
                nc.vector.tensor_copy(
                    vproto[:, :, 0:64],
                    psbv.rearrange("p (h j) -> p h j", h=4),
                )
                nc.vector.tensor_copy(
                    vsb[:],
                    vproto[:, None, :, :].to_broadcast((P, NKB, 4, 65)),
                )

            for _rep in range(repeats):
                # -------- sweep A: K then Q over kt 0..3, kt-major across
                # 8 PSUM accumulators (both jt halves x 4 s-chunks) so the PE
                # tracks the x DMA stream --------
                HKT = KT // 2
                for w_sb, bias_sb, dst in ((wk_sb, bk_sb, kT), (wq_sb, bq_sb, qT)):
                    pa0 = ps_a.tile([P, 2, QC], F32, tag="a")
                    pa1 = ps_a.tile([P, 2, QC], F32, tag="a")
                    po0 = ps_o.tile([P, QC], F32, tag="o")
                    po1 = ps_o.tile([P, QC], F32, tag="o")
                    py0 = ps_yt.tile([P, QC], F32, tag="yt")
                    py1 = ps_yt.tile([P, QC], F32, tag="yt")
                    accs = [
                        pa0[:, 0, :], pa0[:, 1, :], pa1[:, 0, :], pa1[:, 1, :],
                        po0, po1, py0, py1,
                    ]
                    for kt in range(HKT):
                        for g, acc in enumerate(accs):
                            jt, sc = divmod(g, NQC)
                            nc.tensor.matmul(
                                acc,
                                w_sb[:, kt, jt * P : (jt + 1) * P],
                                xsb[:, kt, sc * QC : (sc + 1) * QC],
                                start=(kt == 0),
                                stop=(kt == HKT - 1),
                            )
                    for g, acc in enumerate(accs):
                        jt, sc = divmod(g, NQC)
                        nc.scalar.activation(
                            dst[:, jt, sc * QC : (sc + 1) * QC],
                            acc,
                            AF.Identity,
                            bias=bias_sb[:, jt : jt + 1],
                        )

                # -------- interleaved: per q-chunk, finish the projections
                # this chunk needs, then run its attention; the normalize +
                # out-projection of the previous chunk slot in between so the
                # PE stays fed while the DVE reciprocal chain drains --------
                saved = {}

                def sweep_b(sc):
                    for w_sb, bias_sb, dst in (
                        (wk_sb, bk_sb, kT), (wq_sb, bq_sb, qT)
                    ):
                        for jt in range(2):
                            acc = ps_o.tile([P, QC], F32, tag="o")
                            for kt in range(HKT, KT):
                                nc.tensor.matmul(
                                    acc,
                                    w_sb[:, kt, jt * P : (jt + 1) * P],
                                    xsb[:, kt, sc * QC : (sc + 1) * QC],
                                    start=(kt == HKT),
                                    stop=(kt == KT - 1),
                                )
                            dsl = dst[:, jt, sc * QC : (sc + 1) * QC]
                            nc.vector.tensor_add(dsl, acc, dsl)

                def v_proj(qc):
                    for st_i in range(4 * qc, 4 * qc + 4):
                        acc = ps_yt.tile([P, QC], F32, tag="yt")
                        va = acc[:, 0:JH]
                        for kt in range(KT):
                            nc.tensor.matmul(
                                va,
                                xsb[:, kt, st_i * P : (st_i + 1) * P],
                                wv_sb[:, kt, :],
                                start=(kt == 0),
                                stop=(kt == KT - 1),
                            )
                        vsl = vsb[:, st_i, :, 0:64]
                        nc.vector.tensor_add(
                            vsl, va.rearrange("p (h j) -> p h j", h=4), vsl
                        )

                def attn(qc):
                    nkb = 4 * (qc + 1)
                    for pair in range(2):
                        y0 = ps_yt.tile([P, QC], F32, tag="yt")
                        y1 = ps_yt.tile([P, QC], F32, tag="yt")
                        for kb in range(nkb):
                            d = kb - 4 * qc  # >= 0 on diagonal blocks
                            n_d = QC - 128 * d if d > 0 else QC
                            q_off = qc * QC + (QC - n_d)
                            sc_ps = ps_a.tile([P, 2, QC], F32, tag="a")
                            for he in range(2):
                                nc.tensor.matmul(
                                    sc_ps[:, he, 0:n_d],
                                    kT[64 * he : 64 * he + 64, pair,
                                       kb * P : (kb + 1) * P],
                                    qT[64 * he : 64 * he + 64, pair,
                                       q_off : q_off + n_d],
                                    start=True,
                                    stop=(d < 0),
                                    tile_position=(64 * he, 0),
                                )
                            if d >= 0:
                                # add the causal triangle (cols 0:128 only)
                                for he in range(2):
                                    nc.tensor.matmul(
                                        sc_ps[:, he, 0:P],
                                        ident_b[:],
                                        mask_b[:],
                                        start=False,
                                        stop=True,
                                    )
                            pt = pt_pool.tile([P, 2, QC], BF, tag="pt")
                            nc.scalar.activation(
                                pt[:, :, 0:n_d],
                                sc_ps[:, :, 0:n_d],
                                AF.Exp,
                                scale=0.125,
                            )
                            h0 = 2 * pair
                            for he, yps in ((0, y0), (1, y1)):
                                nc.tensor.matmul(
                                    yps[0:65, QC - n_d :],
                                    vsb[:, kb, h0 + he, :],
                                    pt[:, he, 0:n_d],
                                    start=(kb == 0),
                                    stop=(kb == nkb - 1),
                                )
                        # evacuate AV + sums to SBUF, kick off reciprocals;
                        # the PE-side broadcast + multiply is deferred
                        ysg0 = recip_pool.tile([P, QC], F32, tag="ysg")
                        ysg1 = recip_pool.tile([P, QC], F32, tag="ysg")
                        ymid = recip_pool.tile([P, QC], F32, tag="ysg")
                        nc.vector.tensor_copy(ysg0[0:65, :], y0[0:65, :])
                        nc.vector.tensor_copy(ysg1[0:65, :], y1[0:65, :])
                        with nc.allow_low_precision(
                            reason="f32r reciprocal row for sums broadcast"
                        ):
                            nc.vector.reciprocal(
                                rtr[64:65, pair, 0, :], ysg0[64:65, :]
                            )
                            nc.vector.reciprocal(
                                rtr[64:65, pair, 1, :], ysg1[64:65, :]
                            )
                        # odd head hops to partitions 64:128 for the final mul
                        nc.sync.dma_start(ymid[64:128, :], ysg1[0:64, :])
                        saved[(qc, pair)] = (ysg0, ymid)

                def normalize(qc):
                    for pair in range(2):
                        ysg0, ymid = saved.pop((qc, pair))
                        rbp0 = ps_o.tile([P, QC], F32, tag="o")
                        nc.tensor.matmul(
                            rbp0, e64_r[:], rtr[:, pair, 0, :],
                            start=True, stop=True,
                        )
                        nc.vector.tensor_mul(
                            yT[0:64, pair, qc * QC : (qc + 1) * QC],
                            ysg0[0:64, :],
                            rbp0[0:64, :],
                        )
                        rbp1 = ps_o.tile([P, QC], F32, tag="o")
                        nc.tensor.matmul(
                            rbp1, e64_r[:], rtr[:, pair, 1, :],
                            start=True, stop=True,
                        )
                        nc.vector.tensor_mul(
                            yT[64:128, pair, qc * QC : (qc + 1) * QC],
                            ymid[64:128, :],
                            rbp1[64:128, :],
                        )

                def outproj(qc):
                    for eg in range(4):
                        ot = ostage.tile([P, 2, QC], BF, tag="ot")
                        for sub in range(2):
                            et = 2 * eg + sub
                            acc = ps_o.tile([P, QC], F32, tag="o")
                            for pair in range(2):
                                nc.tensor.matmul(
                                    acc,
                                    wo_sb[:, pair, et * P : (et + 1) * P],
                                    yT[:, pair, qc * QC : (qc + 1) * QC],
                                    start=(pair == 0),
                                    stop=(pair == 1),
                                )
                            nc.vector.tensor_copy(ot[:, sub, :], acc)
                        nc.sync.dma_start(
                            outT_re[:, 2 * eg : 2 * eg + 2,
                                    qc * QC : (qc + 1) * QC],
                            ot[:],
                        )

                for qc in range(NQC):
                    sweep_b(qc)
                    v_proj(qc)
                    if qc > 0:
                        normalize(qc - 1)
                        outproj(qc - 1)
                    attn(qc)
                normalize(NQC - 1)
                outproj(NQC - 1)

    nc.compile()
    return nc


def _get_nc():
    global _NC
    if _NC is None:
        _NC = build()
    return _NC


def core_in_map(x, wq, bq, wk, bk, wv, bv, wo, c):
    """Per-core input dict (core c of 8): batch c//4, head-group c%4."""
    b, g = divmod(c, 4)
    jsl = slice(JH * g, JH * (g + 1))
    return {
        "xT": np.ascontiguousarray(x[b].T).astype(BF16NP),
        "wq": np.ascontiguousarray(wq[:, jsl]).astype(BF16NP),
        "wk": np.ascontiguousarray(wk[:, jsl]).astype(BF16NP),
        "wv": np.ascontiguousarray(wv[:, jsl]).astype(BF16NP),
        "wo": np.ascontiguousarray(wo[jsl, :]).astype(BF16NP),
        "bq": np.ascontiguousarray(bq[jsl]).astype(np.float32),
        "bk": np.ascontiguousarray(bk[jsl]).astype(np.float32),
        "bv": np.ascontiguousarray(bv[jsl]).astype(np.float32),
    }


def kernel(x, wq, bq, wk, bk, wv, bv, wo, bo, **run_kwargs):
    x = np.asarray(x, dtype=np.float32)
    wq = np.asarray(wq, dtype=np.float32)
    bq = np.asarray(bq, dtype=np.float32)
    wk = np.asarray(wk, dtype=np.float32)
    bk = np.asarray(bk, dtype=np.float32)
    wv = np.asarray(wv, dtype=np.float32)
    bv = np.asarray(bv, dtype=np.float32)
    wo = np.asarray(wo, dtype=np.float32)
    bo = np.asarray(bo, dtype=np.float32)

    nc = _get_nc()
    in_maps = [core_in_map(x, wq, bq, wk, bk, wv, bv, wo, c) for c in range(8)]
    res = run_bass_kernel_spmd(nc, in_maps, core_ids=list(range(8)), **run_kwargs)
    outs = [np.asarray(r["outT"], dtype=np.float32) for r in res.results]
    y = np.empty((B, S, D), dtype=np.float32)
    for b in range(B):
        acc = outs[4 * b] + outs[4 * b + 1] + outs[4 * b + 2] + outs[4 * b + 3]
        y[b] = acc.T + bo[None, :]
    if run_kwargs:
        kernel.last_result = res
    return y
